# revision 47
# baseline (speedup 1.0000x reference)
"""AdaConv2D Trainium2 Bass kernel (fp8-DoubleRow + bf16 hybrid conv).

Problem (per sample): instance-norm(x) -> grouped 3x3 conv (128 groups,
2ch/group, per-sample weights) -> grouped 1x1 conv -> +bias.
B=8, Cin=Cout=256, H=W=128.  Pure data-parallel: 1 sample per NeuronCore.

Math: the 1x1 conv folds into the 3x3 taps (w_eff), the instance norm
folds into the weights (scale per in-channel ci) and bias:
    out = W_s @ x_pad + bias',   W_s[ci,t,co] = w_eff * S/(std_ci+eps)
    bias'[co] = bias[co] - (sum_{ci,t} W_s * mean_ci)/S
with x_pad borders held at mean_ci so border windows cancel, and a
global S=128 pre-scale so fp8-quantized weights stay in e4m3's normal
range (the epilogue multiplies by 1/S).

Precision/speed plan (validated vs f64 reference, ~1.5% L2 global,
gate is 2e-2):
  - Host sends x twice, pre-padded to 130x130: xq = fp8 e4m3 (4.3 MiB)
    and xb = bf16 (8.7 MiB).
  - taps 0..5 run as 3 fp8 DoubleRow matmuls on xq (2 taps per
    instruction; DR costs the same per instruction as one bf16 matmul
    but does 2 taps).
  - taps 6..8 run as bf16 matmuls on xb (near-full precision).
  -> 6 PE instructions per psum tile instead of 9 (bf16-only).
  - Output is written bf16 (8 MiB) and upcast to f32 on the host.

Per-core dataflow:
  - xb streams first (10 chunks/half): DVE accumulates sums, ACT
    accumulates sum-of-squares; the xq stream rides the DMA tail.
  - w_eff scatters (via a zero DRAM scratch) into dense block-diag
    [ci, tap, co] layout, loaded back before stats land; after stats a
    DVE pass scales by S/std and quantizes to the fp8/bf16 lhsT tiles.
  - bias' comes from 6 accumulated N=1 matmuls against the fp8/bf16
    mean, mirroring the conv arithmetic exactly (border cancellation).
  - conv: per 16-row superblock, 4 psum tiles x 6 slot-instructions;
    epilogues alternate ACT/DVE (1/S scale + bias'), emit bf16, DMA out.
"""

import sys

sys.path.insert(0, "/opt/trn_rl_repo")

from contextlib import ExitStack

import numpy as np
import ml_dtypes

from concourse import bacc, bass, mybir, tile
from concourse.bass_utils import run_bass_kernel_spmd

F32 = mybir.dt.float32
BF16 = mybir.dt.bfloat16
FP8 = mybir.dt.float8e4
AX = mybir.AxisListType
OP = mybir.AluOpType
ACTF = mybir.ActivationFunctionType
DR = mybir.MatmulPerfMode.DoubleRow

C = 256          # channels (per sample)
H = W = 128      # spatial
P = 128          # partitions
HP = H + 2       # padded rows/cols (130)
NHF = 2          # channel halves
NCHUNK = 10      # input DMA chunks per half (13 padded rows each)
CHUNK_TR = HP // NCHUNK           # 13 tile rows per chunk
ROWS_PER_MM = 4                   # output rows per psum tile (4*128=512)
SB_TILES = 4                      # psum tiles per superblock
SB_ROWS = ROWS_PER_MM * SB_TILES  # 16 rows per superblock
NSB = H // SB_ROWS                # 8 superblocks per half
NPIX = H * W
EPS = 1e-7
S = 128.0        # weight pre-scale (fp8 range), undone in the epilogue

TAPS = [(t // 3, t % 3) for t in range(9)]
FP8_PAIRS = [(0, 1), (2, 3), (4, 5), (6, 7)]  # DoubleRow tap pairs (xq)
BF16_TAPS = [8]                               # bf16 taps (xb)
NPAIR = len(FP8_PAIRS)
NB16 = len(BF16_TAPS)
NSLOT = NPAIR + NB16

_CACHED = {}


def build_nc():
    nc = bacc.Bacc(trn_type="TRN2")

    xq_ext = nc.declare_dram_parameter("xq", [C, HP, HP], FP8, isOutput=False)
    xb_ext = nc.declare_dram_parameter("xb", [C, HP, HP], BF16, isOutput=False)
    dw_ext = nc.declare_dram_parameter("dw_kernels", [C, 2, 3, 3], F32, isOutput=False)
    pw_ext = nc.declare_dram_parameter("pw_kernels", [C, 2, 1, 1], F32, isOutput=False)
    b_ext = nc.declare_dram_parameter("biases", [C], F32, isOutput=False)
    out_ext = nc.declare_dram_parameter("out", [C, H, W], BF16, isOutput=True)

    # zero-initialized DRAM scratch for the dense block-diag w_eff
    # (runtime scatter only writes the fixed nonzero slots -> idempotent).
    # layout [ci, hf, tap, co] f32
    weff_dram = nc.inline_tensor(
        np.zeros((P, NHF, 9, P), dtype=np.float32), name="weff_zero"
    )
    CI_STRIDE = NHF * 9 * P  # 2304 elements per ci row

    with tile.TileContext(nc) as tc, ExitStack() as ctx:
        const_pool = ctx.enter_context(tc.tile_pool(name="const", bufs=1))
        sq_pool = ctx.enter_context(tc.tile_pool(name="sq", bufs=4))
        psum_pool = ctx.enter_context(tc.tile_pool(name="psum", bufs=8, space="PSUM"))
        stage_pool = ctx.enter_context(tc.tile_pool(name="stage", bufs=6))

        # ---------------- persistent tiles ----------------
        xqt = [const_pool.tile([P, HP, HP], FP8, name=f"xqt{hf}") for hf in range(NHF)]
        xbt = [const_pool.tile([P, HP, HP], BF16, name=f"xbt{hf}") for hf in range(NHF)]

        sums = const_pool.tile([P, NHF, NCHUNK], F32, name="sums")
        sumsqs = const_pool.tile([P, NHF, NCHUNK], F32, name="sumsqs")
        st_a = const_pool.tile([P, NHF], F32, name="st_a")
        st_b = const_pool.tile([P, NHF], F32, name="st_b")
        st_c = const_pool.tile([P, NHF], F32, name="st_c")
        mean_ch = const_pool.tile([P, NHF], F32, name="mean_ch")
        mean_bf = const_pool.tile([P, NHF], BF16, name="mean_bf")
        mqt = const_pool.tile([P, NHF, 2, 1], FP8, name="mqt")
        scS = const_pool.tile([P, NHF], F32, name="scS")
        bias_ch = const_pool.tile([P, NHF], F32, name="bias_ch")
        biasp_ch = const_pool.tile([P, NHF], F32, name="biasp_ch")

        # group-layout weights (partition = group)
        dwg = const_pool.tile([P, 2, 2, 9], F32, name="dwg")    # [g, i, j, t]
        pwg = const_pool.tile([P, 2, 2], F32, name="pwg")       # [g, o, i]
        weffg = const_pool.tile([P, 2, 9, 2], F32, name="weffg")  # [g, j, t, o]

        # dense block-diag weights (per-half tiles so half 0's quantize
        # never picks up a false whole-tile dep on half 1's load)
        weffd = [const_pool.tile([P, 9, P], F32, name=f"weffd{h}") for h in range(NHF)]
        wtmp = [const_pool.tile([P, 9, P], F32, name=f"wtmp{h}") for h in range(NHF)]
        wf8 = const_pool.tile([P, NHF, NPAIR, 2, P], FP8, name="wf8")
        wb16 = const_pool.tile([P, NHF, NB16, P], BF16, name="wb16")

        # ACT LUT warm (sqrt/square/identity) off the critical chains
        zz = const_pool.tile([P, 1], F32, name="zz")
        zz2 = const_pool.tile([P, 1], F32, name="zz2")
        with tc.high_priority():
            nc.vector.memset(zz[:], 0.0)
            nc.scalar.activation(out=zz2[:], in_=zz[:], func=ACTF.Square)
            nc.scalar.sqrt(zz2[:], zz[:])
            nc.scalar.activation(
                out=zz2[:], in_=zz[:], func=ACTF.Identity, bias=zz[:], scale=0.0
            )

        # ------------- early DMAs (no stats dependency) -------------
        # weight-path DMAs live on the gpsimd ring so they never queue
        # behind the x stream (sync) or ACT compute (scalar)
        # dwg[g] = dw[2g:2g+2] and pwg[g] = pw[2g:2g+2] are contiguous
        # 144B/16B runs per partition -- single flat descriptors
        nc.gpsimd.dma_start(
            out=dwg[:],
            in_=bass.AP(tensor=dw_ext, offset=0, ap=[[36, P], [1, 36]]),
        )
        nc.gpsimd.dma_start(
            out=pwg[:],
            in_=bass.AP(tensor=pw_ext, offset=0, ap=[[4, P], [1, 4]]),
        )

        # ------------- w_eff (group layout) + scatter + load -------------
        with tc.high_priority():
            for o in range(2):
                nc.vector.tensor_scalar(
                    out=weffg[:, :, :, o],
                    in0=dwg[:, 0],
                    scalar1=pwg[:, o, 0:1],
                    scalar2=None,
                    op0=OP.mult,
                )
                nc.vector.scalar_tensor_tensor(
                    out=weffg[:, :, :, o],
                    in0=dwg[:, 1],
                    scalar=pwg[:, o, 1:2],
                    in1=weffg[:, :, :, o],
                    op0=OP.mult,
                    op1=OP.add,
                )

        def emit_scatter_load(hf, scatter_eng):
            # scatter: dst (ci=2a+j, hf, t, co=2a+o) <- weffg[64*hf + a, o, j, t]
            # one 3-dim DMA per j: dst dims (a, t, o), src dims (a, t, o)
            for j in range(2):
                scatter_eng.dma_start(
                    out=bass.AP(
                        tensor=weff_dram,
                        offset=j * CI_STRIDE + hf * 9 * P,
                        ap=[[2 * CI_STRIDE + 2, 64], [P, 9], [1, 2]],
                    ),
                    in_=bass.AP(
                        tensor=weffg[:].tensor,
                        offset=(64 * hf) * 36 + j * 18,
                        ap=[[36, 64], [2, 9], [1, 2]],
                    ),
                )
            # dense load back: weffd[hf][ci, t, co]
            return nc.gpsimd.dma_start(
                out=weffd[hf][:],
                in_=bass.AP(
                    tensor=weff_dram,
                    offset=hf * 9 * P,
                    ap=[[CI_STRIDE, P], [P, 9], [1, P]],
                ),
            )

        # ------------- x input DMA (xb first, xq rides the tail) -------------
        # 26-row DMA chunks (half the descriptors of the 13-row stats
        # granularity -- the stream is descriptor-rate bound), h0 split
        # across the sync and gpsimd rings
        DC = 2 * CHUNK_TR  # 26 tile rows per DMA chunk
        NDC = HP // DC     # 5 DMA chunks per half per tensor

        def emit_x_dma(eng, ext, tileap, hf, dk):
            r0 = dk * DC
            return eng.dma_start(
                out=tileap[:, r0 : r0 + DC, :],
                in_=bass.AP(
                    tensor=ext,
                    offset=hf * P * HP * HP + r0 * HP,
                    ap=[[HP * HP, P], [1, DC * HP]],
                ),
            )

        with tc.high_priority():
            for dk in range(NDC):
                eng = (nc.sync, nc.gpsimd)[dk % 2]
                xb0_last = emit_x_dma(eng, xb_ext, xbt[0], 0, dk)
            emit_scatter_load(0, nc.gpsimd)
            for dk in range(NDC):
                eng = (nc.sync, nc.gpsimd)[dk % 2]
                emit_x_dma(eng, xq_ext, xqt[0], 0, dk)
            nc.sync.dma_start(
                out=bias_ch[:],
                in_=bass.AP(tensor=b_ext, offset=0, ap=[[1, P], [P, NHF]]),
            )

        # ------------- per-half pipeline -------------
        # Emission order = engine queue order.  Both halves' input/stats/
        # finalize are emitted BEFORE half 0's conv loop so that half 1's
        # ACT/DVE stats ops sit ahead of half 0's epilogues in the queues
        # (they are chunk-paced and finish long before the epilogues need
        # the engines); the PE queue still runs h0 conv -> h1 conv.
        h0_last_dve = None
        last_sq = [None, None]
        h0_sqrt = [None]

        def emit_chunk_stats(hf, ck):
            # per-chunk stats: sums (DVE), sumsq (ACT), both from xb
            r0 = max(1, ck * CHUNK_TR)
            r1 = min(1 + H, (ck + 1) * CHUNK_TR)
            gtr = sq_pool.tile([P, CHUNK_TR, W], BF16, name="gtr")
            ts_inst = nc.vector.tensor_scalar(
                out=gtr[:, 0 : r1 - r0, :],
                in0=xbt[hf][:, r0:r1, 1 : 1 + W],
                scalar1=1.0,
                scalar2=None,
                op0=OP.mult,
                op1=OP.add,
                accum_out=sums[:, hf, ck : ck + 1],
            )
            if hf == 1 and ck == 0 and h0_last_dve is not None:
                bass._add_dep_helper(
                    ts_inst.ins,
                    h0_last_dve.ins,
                    sync=True,
                    reason="keep h1 DVE stats behind h0 weight quantize",
                )
            sq = sq_pool.tile([P, CHUNK_TR, W], BF16, name="sq")
            last_sq[hf] = nc.scalar.activation(
                out=sq[:, 0 : r1 - r0, :],
                in_=xbt[hf][:, r0:r1, 1 : 1 + W],
                func=ACTF.Square,
                accum_out=sumsqs[:, hf, ck : ck + 1],
            )
            if hf == 1 and ck == 0 and h0_sqrt[0] is not None:
                bass._add_dep_helper(
                    last_sq[hf].ins,
                    h0_sqrt[0].ins,
                    sync=True,
                    reason="keep h1 ACT stats behind h0 sqrt",
                )

        def emit_h1_dmas():
            # h1 xb split over both rings, gated on the end of h0's xb
            # stream; h1 xq rides last
            for dk in range(NDC):
                eng = (nc.gpsimd, nc.sync)[dk % 2]
                inst = emit_x_dma(eng, xb_ext, xbt[1], 1, dk)
                if dk in (0, 1):
                    bass._add_dep_helper(
                        inst.ins,
                        xb0_last.ins,
                        sync=True,
                        reason="h1 xb stream waits for h0 xb stream",
                    )
            for dk in range(NDC):
                eng = (nc.gpsimd, nc.sync)[dk % 2]
                emit_x_dma(eng, xq_ext, xqt[1], 1, dk)
            emit_scatter_load(1, nc.gpsimd)

        def emit_finalize(hf):
            nonlocal h0_last_dve
            # --- stats finalize (h0's is latency-critical; h1's must NOT
            # be hoisted above h0's quantize in the DVE queue) ---
            from contextlib import nullcontext

            with tc.high_priority() if hf == 0 else nullcontext():
                nc.vector.tensor_reduce(
                    out=st_a[:, hf : hf + 1], in_=sums[:, hf, :], axis=AX.X, op=OP.add
                )
                nc.vector.tensor_scalar(
                    out=mean_ch[:, hf : hf + 1],
                    in0=st_a[:, hf : hf + 1],
                    scalar1=1.0 / NPIX,
                    scalar2=None,
                    op0=OP.mult,
                )
                nc.vector.tensor_reduce(
                    out=st_b[:, hf : hf + 1], in_=sumsqs[:, hf, :], axis=AX.X, op=OP.add
                )
                nc.vector.tensor_tensor(
                    out=st_c[:, hf : hf + 1],
                    in0=mean_ch[:, hf : hf + 1],
                    in1=mean_ch[:, hf : hf + 1],
                    op=OP.mult,
                )
                nc.vector.scalar_tensor_tensor(
                    out=st_b[:, hf : hf + 1],
                    in0=st_c[:, hf : hf + 1],
                    scalar=float(-NPIX),
                    in1=st_b[:, hf : hf + 1],
                    op0=OP.mult,
                    op1=OP.add,
                )
                nc.vector.tensor_scalar(
                    out=st_b[:, hf : hf + 1],
                    in0=st_b[:, hf : hf + 1],
                    scalar1=1.0 / (NPIX - 1),
                    scalar2=None,
                    op0=OP.mult,
                )
                sq_i = nc.scalar.sqrt(st_b[:, hf : hf + 1], st_b[:, hf : hf + 1])
                if hf == 0:
                    h0_sqrt[0] = sq_i
                # (std + EPS) / S, then reciprocal -> S/(std+EPS)
                nc.vector.tensor_scalar(
                    out=st_b[:, hf : hf + 1],
                    in0=st_b[:, hf : hf + 1],
                    scalar1=EPS,
                    scalar2=1.0 / S,
                    op0=OP.add,
                    op1=OP.mult,
                )
                nc.vector.reciprocal(scS[:, hf : hf + 1], st_b[:, hf : hf + 1])
                nc.vector.tensor_copy(mean_bf[:, hf : hf + 1], mean_ch[:, hf : hf + 1])
                nc.vector.tensor_copy(mqt[:, hf, 0], mean_ch[:, hf : hf + 1])
                nc.vector.tensor_copy(mqt[:, hf, 1], mean_ch[:, hf : hf + 1])

                # --- scale + quantize the dense weights ---
                nc.vector.tensor_scalar(
                    out=wtmp[hf][:],
                    in0=weffd[hf][:],
                    scalar1=scS[:, hf : hf + 1],
                    scalar2=None,
                    op0=OP.mult,
                )
                # fp8 taps -> wf8[hf] ([P, NPAIR*2, P] contiguous)
                nc.vector.tensor_copy(
                    bass.AP(
                        tensor=wf8[:].tensor,
                        offset=hf * NPAIR * 2 * P,
                        ap=[[NHF * NPAIR * 2 * P, P], [P, NPAIR * 2], [1, P]],
                    ),
                    wtmp[hf][:, 0 : 2 * NPAIR, :],
                )
                # bf16 taps
                h0_last_dve = nc.vector.tensor_copy(
                    wb16[:, hf], wtmp[hf][:, 2 * NPAIR : 9, :]
                )

            # (x keeps its host-written zero borders: the resulting border
            # error is -sum_padtaps ws*mean ~ 0.1% global -- negligible)

        def emit_conv(hf, interleave=None):
            # --- bias' = bias - (W_s @ mean)/S  (accumulated N=1 matmuls) ---
            bps = psum_pool.tile([P, 1], F32, name="bps", tag="ps", bufs=4)
            si = 0
            for p in range(NPAIR):
                nc.tensor.matmul(
                    bps[:],
                    lhsT=wf8[:, hf, p],
                    rhs=mqt[:, hf],
                    start=(si == 0),
                    stop=(si == NSLOT - 1),
                    perf_mode=DR,
                )
                si += 1
            for i in range(NB16):
                nc.tensor.matmul(
                    bps[:],
                    lhsT=wb16[:, hf, i],
                    rhs=mean_bf[:, hf : hf + 1],
                    start=(si == 0),
                    stop=(si == NSLOT - 1),
                )
                si += 1
            nc.vector.scalar_tensor_tensor(
                out=biasp_ch[:, hf : hf + 1],
                in0=bps[:],
                scalar=-1.0 / S,
                in1=bias_ch[:, hf : hf + 1],
                op0=OP.mult,
                op1=OP.add,
            )

            # --- conv: per superblock, 5 slot-instructions x 4 psum tiles
            # (as 2 double-bank tiles, so each epilogue drains 2 banks) ---
            XPITCH = HP * HP  # xqt partition pitch (elements)
            for sb in range(NSB):
                if interleave is not None:
                    interleave(sb)
                ps2 = [
                    psum_pool.tile(
                        [P, 2, ROWS_PER_MM, W], F32, name="ps", tag="ps", bufs=4
                    )
                    for _ in range(SB_TILES // 2)
                ]
                ps = [ps2[k // 2][:, k % 2] for k in range(SB_TILES)]
                si = 0
                for p, (t0, t1) in enumerate(FP8_PAIRS):
                    dy0, dx0 = TAPS[t0]
                    dy1, dx1 = TAPS[t1]
                    delta = (dy1 - dy0) * HP + (dx1 - dx0)
                    for k in range(SB_TILES):
                        h0 = sb * SB_ROWS + k * ROWS_PER_MM
                        rhs = bass.AP(
                            tensor=xqt[hf][:].tensor,
                            offset=(h0 + dy0) * HP + dx0,
                            ap=[[XPITCH, P], [delta, 2], [HP, ROWS_PER_MM], [1, W]],
                        )
                        nc.tensor.matmul(
                            ps[k],
                            lhsT=wf8[:, hf, p],
                            rhs=rhs,
                            start=(si == 0),
                            stop=(si == NSLOT - 1),
                            perf_mode=DR,
                        )
                    si += 1
                for i, t in enumerate(BF16_TAPS):
                    dy, dx = TAPS[t]
                    for k in range(SB_TILES):
                        h0 = sb * SB_ROWS + k * ROWS_PER_MM
                        nc.tensor.matmul(
                            ps[k],
                            lhsT=wb16[:, hf, i],
                            rhs=xbt[hf][:, h0 + dy : h0 + dy + ROWS_PER_MM, dx : dx + W],
                            start=(si == 0),
                            stop=(si == NSLOT - 1),
                        )
                    si += 1
                # epilogue + store in 8-row blocks (one 2-bank psum tile
                # each); alternate ACT/DVE so neither engine bottlenecks
                for half_blk in range(2):
                    stg = stage_pool.tile([P, SB_ROWS // 2, W], BF16, name="stg")
                    if half_blk == 0:
                        nc.scalar.activation(
                            out=stg[:],
                            in_=ps2[half_blk][:],
                            func=ACTF.Identity,
                            bias=biasp_ch[:, hf : hf + 1],
                            scale=1.0 / S,
                        )
                    else:
                        nc.vector.tensor_scalar(
                            out=stg[:],
                            in0=ps2[half_blk][:],
                            scalar1=1.0 / S,
                            scalar2=biasp_ch[:, hf : hf + 1],
                            op0=OP.mult,
                            op1=OP.add,
                        )
                    out_eng = (nc.gpsimd, nc.sync)[(sb * 2 + half_blk) % 2]
                    out_eng.dma_start(
                        out=out_ext[
                            hf * P : (hf + 1) * P,
                            sb * SB_ROWS
                            + half_blk * (SB_ROWS // 2) : sb * SB_ROWS
                            + (half_blk + 1) * (SB_ROWS // 2),
                            :,
                        ],
                        in_=stg[:],
                    )

        for ck in range(NCHUNK):
            emit_chunk_stats(0, ck)
        emit_finalize(0)
        emit_h1_dmas()

        # interleave h1's chunk-paced stats between h0's conv superblocks
        # so h0's epilogues are never queued behind a long stats stream
        def interleave(sb):
            lo = NCHUNK * sb // (NSB - 1)
            hi = NCHUNK * (sb + 1) // (NSB - 1)
            for ck in range(lo, min(hi, NCHUNK)):
                emit_chunk_stats(1, ck)
            if hi >= NCHUNK and lo < NCHUNK:
                emit_finalize(1)

        emit_conv(0, interleave=interleave)
        emit_conv(1)

    nc.compile()
    return nc


def get_nc():
    if "nc" not in _CACHED:
        _CACHED["nc"] = build_nc()
    return _CACHED["nc"]


def make_in_maps(x, dw_kernels, pw_kernels, biases):
    x = np.asarray(x, dtype=np.float32)
    dw_kernels = np.asarray(dw_kernels, dtype=np.float32)
    pw_kernels = np.asarray(pw_kernels, dtype=np.float32)
    biases = np.asarray(biases, dtype=np.float32)
    B = x.shape[0]
    in_maps = []
    for i in range(B):
        xq = np.zeros((C, HP, HP), dtype=ml_dtypes.float8_e4m3)
        xb = np.zeros((C, HP, HP), dtype=ml_dtypes.bfloat16)
        xq[:, 1 : 1 + H, 1 : 1 + W] = x[i].astype(ml_dtypes.float8_e4m3)
        xb[:, 1 : 1 + H, 1 : 1 + W] = x[i].astype(ml_dtypes.bfloat16)
        in_maps.append(
            {
                "xq": xq,
                "xb": xb,
                "dw_kernels": np.ascontiguousarray(dw_kernels[i]),
                "pw_kernels": np.ascontiguousarray(pw_kernels[i]),
                "biases": np.ascontiguousarray(biases[i]),
            }
        )
    return in_maps


def postprocess(res, B):
    return np.stack(
        [np.asarray(res.results[i]["out"]).astype(np.float32) for i in range(B)], axis=0
    )


def kernel(x, dw_kernels, pw_kernels, biases):
    B = np.asarray(x).shape[0]
    assert B == 8
    nc = get_nc()
    in_maps = make_in_maps(x, dw_kernels, pw_kernels, biases)
    res = run_bass_kernel_spmd(nc, in_maps, core_ids=list(range(B)))
    return postprocess(res, B)


# revision 48
# speedup vs baseline: 1.0343x; 1.0343x over previous
"""AdaConv2D Trainium2 Bass kernel (fp8-DoubleRow + bf16 hybrid conv).

Problem (per sample): instance-norm(x) -> grouped 3x3 conv (128 groups,
2ch/group, per-sample weights) -> grouped 1x1 conv -> +bias.
B=8, Cin=Cout=256, H=W=128.  Pure data-parallel: 1 sample per NeuronCore.

Math: the 1x1 conv folds into the 3x3 taps (w_eff), the instance norm
folds into the weights (scale per in-channel ci) and bias:
    out = W_s @ x_pad + bias',   W_s[ci,t,co] = w_eff * S/(std_ci+eps)
    bias'[co] = bias[co] - (sum_{ci,t} W_s * mean_ci)/S
with x_pad borders held at mean_ci so border windows cancel, and a
global S=128 pre-scale so fp8-quantized weights stay in e4m3's normal
range (the epilogue multiplies by 1/S).

Precision/speed plan (validated vs f64 reference, ~1.5% L2 global,
gate is 2e-2):
  - Host sends x twice, pre-padded to 130x130: xq = fp8 e4m3 (4.3 MiB)
    and xb = bf16 (8.7 MiB).
  - taps 0..5 run as 3 fp8 DoubleRow matmuls on xq (2 taps per
    instruction; DR costs the same per instruction as one bf16 matmul
    but does 2 taps).
  - taps 6..8 run as bf16 matmuls on xb (near-full precision).
  -> 6 PE instructions per psum tile instead of 9 (bf16-only).
  - Output is written bf16 (8 MiB) and upcast to f32 on the host.

Per-core dataflow:
  - xb streams first (10 chunks/half): DVE accumulates sums, ACT
    accumulates sum-of-squares; the xq stream rides the DMA tail.
  - w_eff scatters (via a zero DRAM scratch) into dense block-diag
    [ci, tap, co] layout, loaded back before stats land; after stats a
    DVE pass scales by S/std and quantizes to the fp8/bf16 lhsT tiles.
  - bias' comes from 6 accumulated N=1 matmuls against the fp8/bf16
    mean, mirroring the conv arithmetic exactly (border cancellation).
  - conv: per 16-row superblock, 4 psum tiles x 6 slot-instructions;
    epilogues alternate ACT/DVE (1/S scale + bias'), emit bf16, DMA out.
"""

import sys

sys.path.insert(0, "/opt/trn_rl_repo")

from contextlib import ExitStack

import numpy as np
import ml_dtypes

from concourse import bacc, bass, mybir, tile
from concourse.bass_utils import run_bass_kernel_spmd

F32 = mybir.dt.float32
BF16 = mybir.dt.bfloat16
FP8 = mybir.dt.float8e4
AX = mybir.AxisListType
OP = mybir.AluOpType
ACTF = mybir.ActivationFunctionType
DR = mybir.MatmulPerfMode.DoubleRow

C = 256          # channels (per sample)
H = W = 128      # spatial
P = 128          # partitions
HP = H + 2       # padded rows/cols (130)
NHF = 2          # channel halves
NCHUNK = 10      # input DMA chunks per half (13 padded rows each)
CHUNK_TR = HP // NCHUNK           # 13 tile rows per chunk
ROWS_PER_MM = 4                   # output rows per psum tile (4*128=512)
SB_TILES = 4                      # psum tiles per superblock
SB_ROWS = ROWS_PER_MM * SB_TILES  # 16 rows per superblock
NSB = H // SB_ROWS                # 8 superblocks per half
NPIX = H * W
EPS = 1e-7
S = 128.0        # weight pre-scale (fp8 range), undone in the epilogue

TAPS = [(t // 3, t % 3) for t in range(9)]
FP8_PAIRS = [(0, 1), (2, 3), (4, 5), (6, 7)]  # DoubleRow tap pairs (xq)
BF16_TAPS = [8]                               # bf16 taps (xb)
NPAIR = len(FP8_PAIRS)
NB16 = len(BF16_TAPS)
NSLOT = NPAIR + NB16

_CACHED = {}


def build_nc():
    nc = bacc.Bacc(trn_type="TRN2")

    xq_ext = nc.declare_dram_parameter("xq", [C, HP, HP], FP8, isOutput=False)
    xb_ext = nc.declare_dram_parameter("xb", [C, HP, HP], BF16, isOutput=False)
    dw_ext = nc.declare_dram_parameter("dw_kernels", [C, 2, 3, 3], F32, isOutput=False)
    pw_ext = nc.declare_dram_parameter("pw_kernels", [C, 2, 1, 1], F32, isOutput=False)
    b_ext = nc.declare_dram_parameter("biases", [C], F32, isOutput=False)
    out_ext = nc.declare_dram_parameter("out", [C, H, W], BF16, isOutput=True)

    # zero-initialized DRAM scratch for the dense block-diag w_eff
    # (runtime scatter only writes the fixed nonzero slots -> idempotent).
    # layout [ci, hf, tap, co] f32
    weff_dram = nc.inline_tensor(
        np.zeros((P, NHF, 9, P), dtype=np.float32), name="weff_zero"
    )
    CI_STRIDE = NHF * 9 * P  # 2304 elements per ci row

    with tile.TileContext(nc) as tc, ExitStack() as ctx:
        const_pool = ctx.enter_context(tc.tile_pool(name="const", bufs=1))
        sq_pool = ctx.enter_context(tc.tile_pool(name="sq", bufs=4))
        psum_pool = ctx.enter_context(tc.tile_pool(name="psum", bufs=8, space="PSUM"))
        stage_pool = ctx.enter_context(tc.tile_pool(name="stage", bufs=6))

        # ---------------- persistent tiles ----------------
        xqt = [const_pool.tile([P, HP, HP], FP8, name=f"xqt{hf}") for hf in range(NHF)]
        xbt = [const_pool.tile([P, HP, HP], BF16, name=f"xbt{hf}") for hf in range(NHF)]

        sums = const_pool.tile([P, NHF, NCHUNK], F32, name="sums")
        sumsqs = const_pool.tile([P, NHF, NCHUNK], F32, name="sumsqs")
        st_a = const_pool.tile([P, NHF], F32, name="st_a")
        st_b = const_pool.tile([P, NHF], F32, name="st_b")
        st_c = const_pool.tile([P, NHF], F32, name="st_c")
        mean_ch = const_pool.tile([P, NHF], F32, name="mean_ch")
        mean_bf = const_pool.tile([P, NHF], BF16, name="mean_bf")
        mqt = const_pool.tile([P, NHF, 2, 1], FP8, name="mqt")
        scS = const_pool.tile([P, NHF], F32, name="scS")
        bias_ch = const_pool.tile([P, NHF], F32, name="bias_ch")
        biasp_ch = const_pool.tile([P, NHF], F32, name="biasp_ch")

        # group-layout weights (partition = group)
        dwg = const_pool.tile([P, 2, 2, 9], F32, name="dwg")    # [g, i, j, t]
        pwg = const_pool.tile([P, 2, 2], F32, name="pwg")       # [g, o, i]
        weffg = const_pool.tile([P, 2, 9, 2], F32, name="weffg")  # [g, j, t, o]

        # dense block-diag weights (per-half tiles so half 0's quantize
        # never picks up a false whole-tile dep on half 1's load)
        weffd = [const_pool.tile([P, 9, P], F32, name=f"weffd{h}") for h in range(NHF)]
        wtmp = [const_pool.tile([P, 9, P], F32, name=f"wtmp{h}") for h in range(NHF)]
        wf8 = const_pool.tile([P, NHF, NPAIR, 2, P], FP8, name="wf8")
        wb16 = const_pool.tile([P, NHF, NB16, P], BF16, name="wb16")

        # ACT LUT warm (sqrt/square/identity) off the critical chains
        zz = const_pool.tile([P, 1], F32, name="zz")
        zz2 = const_pool.tile([P, 1], F32, name="zz2")
        with tc.high_priority():
            nc.vector.memset(zz[:], 0.0)
            nc.scalar.activation(out=zz2[:], in_=zz[:], func=ACTF.Square)
            nc.scalar.sqrt(zz2[:], zz[:])
            nc.scalar.activation(
                out=zz2[:], in_=zz[:], func=ACTF.Identity, bias=zz[:], scale=0.0
            )

        # ------------- early DMAs (no stats dependency) -------------
        # weight-path DMAs live on the gpsimd ring so they never queue
        # behind the x stream (sync) or ACT compute (scalar)
        # dwg[g] = dw[2g:2g+2] and pwg[g] = pw[2g:2g+2] are contiguous
        # 144B/16B runs per partition -- single flat descriptors
        nc.gpsimd.dma_start(
            out=dwg[:],
            in_=bass.AP(tensor=dw_ext, offset=0, ap=[[36, P], [1, 36]]),
        )
        nc.gpsimd.dma_start(
            out=pwg[:],
            in_=bass.AP(tensor=pw_ext, offset=0, ap=[[4, P], [1, 4]]),
        )

        # ------------- w_eff (group layout) + scatter + load -------------
        with tc.high_priority():
            for o in range(2):
                nc.vector.tensor_scalar(
                    out=weffg[:, :, :, o],
                    in0=dwg[:, 0],
                    scalar1=pwg[:, o, 0:1],
                    scalar2=None,
                    op0=OP.mult,
                )
                nc.vector.scalar_tensor_tensor(
                    out=weffg[:, :, :, o],
                    in0=dwg[:, 1],
                    scalar=pwg[:, o, 1:2],
                    in1=weffg[:, :, :, o],
                    op0=OP.mult,
                    op1=OP.add,
                )

        def emit_scatter_load(hf, scatter_eng):
            # scatter: dst (ci=2a+j, hf, t, co=2a+o) <- weffg[64*hf + a, o, j, t]
            # one 3-dim DMA per j: dst dims (a, t, o), src dims (a, t, o)
            for j in range(2):
                scatter_eng.dma_start(
                    out=bass.AP(
                        tensor=weff_dram,
                        offset=j * CI_STRIDE + hf * 9 * P,
                        ap=[[2 * CI_STRIDE + 2, 64], [P, 9], [1, 2]],
                    ),
                    in_=bass.AP(
                        tensor=weffg[:].tensor,
                        offset=(64 * hf) * 36 + j * 18,
                        ap=[[36, 64], [2, 9], [1, 2]],
                    ),
                )
            # dense load back: weffd[hf][ci, t, co]
            return nc.gpsimd.dma_start(
                out=weffd[hf][:],
                in_=bass.AP(
                    tensor=weff_dram,
                    offset=hf * 9 * P,
                    ap=[[CI_STRIDE, P], [P, 9], [1, P]],
                ),
            )

        # ------------- x input DMA (xb first, xq rides the tail) -------------
        # 26-row DMA chunks (half the descriptors of the 13-row stats
        # granularity -- the stream is descriptor-rate bound), h0 split
        # across the sync and gpsimd rings
        DC = 2 * CHUNK_TR  # 26 tile rows per DMA chunk
        NDC = HP // DC     # 5 DMA chunks per half per tensor

        def emit_x_dma(eng, ext, tileap, hf, dk):
            r0 = dk * DC
            return eng.dma_start(
                out=tileap[:, r0 : r0 + DC, :],
                in_=bass.AP(
                    tensor=ext,
                    offset=hf * P * HP * HP + r0 * HP,
                    ap=[[HP * HP, P], [1, DC * HP]],
                ),
            )

        with tc.high_priority():
            for dk in range(NDC):
                eng = (nc.sync, nc.gpsimd)[dk % 2]
                xb0_last = emit_x_dma(eng, xb_ext, xbt[0], 0, dk)
            emit_scatter_load(0, nc.gpsimd)
            for dk in range(NDC):
                eng = (nc.sync, nc.gpsimd)[dk % 2]
                emit_x_dma(eng, xq_ext, xqt[0], 0, dk)
            nc.sync.dma_start(
                out=bias_ch[:],
                in_=bass.AP(tensor=b_ext, offset=0, ap=[[1, P], [P, NHF]]),
            )

        # ------------- per-half pipeline -------------
        # Emission order = engine queue order.  Both halves' input/stats/
        # finalize are emitted BEFORE half 0's conv loop so that half 1's
        # ACT/DVE stats ops sit ahead of half 0's epilogues in the queues
        # (they are chunk-paced and finish long before the epilogues need
        # the engines); the PE queue still runs h0 conv -> h1 conv.
        h0_last_dve = None
        last_sq = [None, None]
        h0_sqrt = [None]

        def emit_chunk_stats(hf, ck):
            # per-chunk stats: sums (DVE), sumsq (ACT), both from xb
            r0 = max(1, ck * CHUNK_TR)
            r1 = min(1 + H, (ck + 1) * CHUNK_TR)
            gtr = sq_pool.tile([P, CHUNK_TR, W], BF16, name="gtr")
            ts_inst = nc.vector.tensor_scalar(
                out=gtr[:, 0 : r1 - r0, :],
                in0=xbt[hf][:, r0:r1, 1 : 1 + W],
                scalar1=1.0,
                scalar2=None,
                op0=OP.mult,
                op1=OP.add,
                accum_out=sums[:, hf, ck : ck + 1],
            )
            if hf == 1 and ck == 0 and h0_last_dve is not None:
                bass._add_dep_helper(
                    ts_inst.ins,
                    h0_last_dve.ins,
                    sync=True,
                    reason="keep h1 DVE stats behind h0 weight quantize",
                )
            sq = sq_pool.tile([P, CHUNK_TR, W], BF16, name="sq")
            last_sq[hf] = nc.scalar.activation(
                out=sq[:, 0 : r1 - r0, :],
                in_=xbt[hf][:, r0:r1, 1 : 1 + W],
                func=ACTF.Square,
                accum_out=sumsqs[:, hf, ck : ck + 1],
            )
            if hf == 1 and ck == 0 and h0_sqrt[0] is not None:
                bass._add_dep_helper(
                    last_sq[hf].ins,
                    h0_sqrt[0].ins,
                    sync=True,
                    reason="keep h1 ACT stats behind h0 sqrt",
                )

        def emit_h1_dmas():
            # h1 xb split over both rings, gated on the end of h0's xb
            # stream; h1 xq rides last
            for dk in range(NDC):
                eng = (nc.gpsimd, nc.sync)[dk % 2]
                inst = emit_x_dma(eng, xb_ext, xbt[1], 1, dk)
                if dk in (0, 1):
                    bass._add_dep_helper(
                        inst.ins,
                        xb0_last.ins,
                        sync=True,
                        reason="h1 xb stream waits for h0 xb stream",
                    )
            for dk in range(NDC):
                eng = (nc.gpsimd, nc.sync)[dk % 2]
                emit_x_dma(eng, xq_ext, xqt[1], 1, dk)
            emit_scatter_load(1, nc.gpsimd)

        def emit_finalize(hf):
            nonlocal h0_last_dve
            # --- stats finalize (h0's is latency-critical; h1's must NOT
            # be hoisted above h0's quantize in the DVE queue) ---
            from contextlib import nullcontext

            with tc.high_priority() if hf == 0 else nullcontext():
                nc.vector.tensor_reduce(
                    out=st_a[:, hf : hf + 1], in_=sums[:, hf, :], axis=AX.X, op=OP.add
                )
                nc.vector.tensor_scalar(
                    out=mean_ch[:, hf : hf + 1],
                    in0=st_a[:, hf : hf + 1],
                    scalar1=1.0 / NPIX,
                    scalar2=None,
                    op0=OP.mult,
                )
                nc.vector.tensor_reduce(
                    out=st_b[:, hf : hf + 1], in_=sumsqs[:, hf, :], axis=AX.X, op=OP.add
                )
                nc.vector.tensor_tensor(
                    out=st_c[:, hf : hf + 1],
                    in0=mean_ch[:, hf : hf + 1],
                    in1=mean_ch[:, hf : hf + 1],
                    op=OP.mult,
                )
                nc.vector.scalar_tensor_tensor(
                    out=st_b[:, hf : hf + 1],
                    in0=st_c[:, hf : hf + 1],
                    scalar=float(-NPIX),
                    in1=st_b[:, hf : hf + 1],
                    op0=OP.mult,
                    op1=OP.add,
                )
                nc.vector.tensor_scalar(
                    out=st_b[:, hf : hf + 1],
                    in0=st_b[:, hf : hf + 1],
                    scalar1=1.0 / (NPIX - 1),
                    scalar2=None,
                    op0=OP.mult,
                )
                sq_i = nc.scalar.sqrt(st_b[:, hf : hf + 1], st_b[:, hf : hf + 1])
                if hf == 0:
                    h0_sqrt[0] = sq_i
                # (std + EPS) / S, then reciprocal -> S/(std+EPS)
                nc.vector.tensor_scalar(
                    out=st_b[:, hf : hf + 1],
                    in0=st_b[:, hf : hf + 1],
                    scalar1=EPS,
                    scalar2=1.0 / S,
                    op0=OP.add,
                    op1=OP.mult,
                )
                nc.vector.reciprocal(scS[:, hf : hf + 1], st_b[:, hf : hf + 1])
                nc.vector.tensor_copy(mean_bf[:, hf : hf + 1], mean_ch[:, hf : hf + 1])
                nc.vector.tensor_copy(mqt[:, hf, 0], mean_ch[:, hf : hf + 1])
                nc.vector.tensor_copy(mqt[:, hf, 1], mean_ch[:, hf : hf + 1])

                # --- scale + quantize the dense weights ---
                nc.vector.tensor_scalar(
                    out=wtmp[hf][:],
                    in0=weffd[hf][:],
                    scalar1=scS[:, hf : hf + 1],
                    scalar2=None,
                    op0=OP.mult,
                )
                # fp8 taps -> wf8[hf] ([P, NPAIR*2, P] contiguous)
                nc.vector.tensor_copy(
                    bass.AP(
                        tensor=wf8[:].tensor,
                        offset=hf * NPAIR * 2 * P,
                        ap=[[NHF * NPAIR * 2 * P, P], [P, NPAIR * 2], [1, P]],
                    ),
                    wtmp[hf][:, 0 : 2 * NPAIR, :],
                )
                # bf16 taps
                h0_last_dve = nc.vector.tensor_copy(
                    wb16[:, hf], wtmp[hf][:, 2 * NPAIR : 9, :]
                )

            # (x keeps its host-written zero borders: the resulting border
            # error is -sum_padtaps ws*mean ~ 0.1% global -- negligible)

        def emit_conv(hf, interleave=None):
            # --- bias' = bias - (W_s @ mean)/S  (accumulated N=1 matmuls) ---
            bps = psum_pool.tile([P, 1], F32, name="bps", tag="ps", bufs=4)
            si = 0
            for p in range(NPAIR):
                nc.tensor.matmul(
                    bps[:],
                    lhsT=wf8[:, hf, p],
                    rhs=mqt[:, hf],
                    start=(si == 0),
                    stop=(si == NSLOT - 1),
                    perf_mode=DR,
                )
                si += 1
            for i in range(NB16):
                nc.tensor.matmul(
                    bps[:],
                    lhsT=wb16[:, hf, i],
                    rhs=mean_bf[:, hf : hf + 1],
                    start=(si == 0),
                    stop=(si == NSLOT - 1),
                )
                si += 1
            nc.vector.scalar_tensor_tensor(
                out=biasp_ch[:, hf : hf + 1],
                in0=bps[:],
                scalar=-1.0 / S,
                in1=bias_ch[:, hf : hf + 1],
                op0=OP.mult,
                op1=OP.add,
            )

            # --- conv: per superblock, 5 slot-instructions x 4 psum tiles
            # (as 2 double-bank tiles, so each epilogue drains 2 banks) ---
            XPITCH = HP * HP  # xqt partition pitch (elements)
            for sb in range(NSB):
                if interleave is not None:
                    interleave(sb)
                ps2 = [
                    psum_pool.tile(
                        [P, 2, ROWS_PER_MM, W], F32, name="ps", tag="ps", bufs=4
                    )
                    for _ in range(SB_TILES // 2)
                ]
                ps = [ps2[k // 2][:, k % 2] for k in range(SB_TILES)]
                si = 0
                for p, (t0, t1) in enumerate(FP8_PAIRS):
                    dy0, dx0 = TAPS[t0]
                    dy1, dx1 = TAPS[t1]
                    delta = (dy1 - dy0) * HP + (dx1 - dx0)
                    for k in range(SB_TILES):
                        h0 = sb * SB_ROWS + k * ROWS_PER_MM
                        rhs = bass.AP(
                            tensor=xqt[hf][:].tensor,
                            offset=(h0 + dy0) * HP + dx0,
                            ap=[[XPITCH, P], [delta, 2], [HP, ROWS_PER_MM], [1, W]],
                        )
                        nc.tensor.matmul(
                            ps[k],
                            lhsT=wf8[:, hf, p],
                            rhs=rhs,
                            start=(si == 0),
                            stop=(si == NSLOT - 1),
                            perf_mode=DR,
                        )
                    si += 1
                for i, t in enumerate(BF16_TAPS):
                    dy, dx = TAPS[t]
                    for k in range(SB_TILES):
                        h0 = sb * SB_ROWS + k * ROWS_PER_MM
                        nc.tensor.matmul(
                            ps[k],
                            lhsT=wb16[:, hf, i],
                            rhs=xbt[hf][:, h0 + dy : h0 + dy + ROWS_PER_MM, dx : dx + W],
                            start=(si == 0),
                            stop=(si == NSLOT - 1),
                        )
                    si += 1
                # epilogue + store in 8-row blocks (one 2-bank psum tile
                # each); alternate ACT/DVE so neither engine bottlenecks
                for half_blk in range(2):
                    stg = stage_pool.tile([P, SB_ROWS // 2, W], BF16, name="stg")
                    if half_blk == 0:
                        nc.scalar.activation(
                            out=stg[:],
                            in_=ps2[half_blk][:],
                            func=ACTF.Identity,
                            bias=biasp_ch[:, hf : hf + 1],
                            scale=1.0 / S,
                        )
                    else:
                        nc.vector.tensor_scalar(
                            out=stg[:],
                            in0=ps2[half_blk][:],
                            scalar1=1.0 / S,
                            scalar2=biasp_ch[:, hf : hf + 1],
                            op0=OP.mult,
                            op1=OP.add,
                        )
                    out_eng = (nc.gpsimd, nc.sync)[(sb * 2 + half_blk) % 2]
                    out_eng.dma_start(
                        out=out_ext[
                            hf * P : (hf + 1) * P,
                            sb * SB_ROWS
                            + half_blk * (SB_ROWS // 2) : sb * SB_ROWS
                            + (half_blk + 1) * (SB_ROWS // 2),
                            :,
                        ],
                        in_=stg[:],
                    )

        for ck in range(NCHUNK):
            emit_chunk_stats(0, ck)
        emit_finalize(0)
        emit_h1_dmas()
        for ck in range(NCHUNK):
            emit_chunk_stats(1, ck)
        emit_finalize(1)
        emit_conv(0)
        emit_conv(1)

    nc.compile()
    return nc


def get_nc():
    if "nc" not in _CACHED:
        _CACHED["nc"] = build_nc()
    return _CACHED["nc"]


def make_in_maps(x, dw_kernels, pw_kernels, biases):
    x = np.asarray(x, dtype=np.float32)
    dw_kernels = np.asarray(dw_kernels, dtype=np.float32)
    pw_kernels = np.asarray(pw_kernels, dtype=np.float32)
    biases = np.asarray(biases, dtype=np.float32)
    B = x.shape[0]
    in_maps = []
    for i in range(B):
        xq = np.zeros((C, HP, HP), dtype=ml_dtypes.float8_e4m3)
        xb = np.zeros((C, HP, HP), dtype=ml_dtypes.bfloat16)
        xq[:, 1 : 1 + H, 1 : 1 + W] = x[i].astype(ml_dtypes.float8_e4m3)
        xb[:, 1 : 1 + H, 1 : 1 + W] = x[i].astype(ml_dtypes.bfloat16)
        in_maps.append(
            {
                "xq": xq,
                "xb": xb,
                "dw_kernels": np.ascontiguousarray(dw_kernels[i]),
                "pw_kernels": np.ascontiguousarray(pw_kernels[i]),
                "biases": np.ascontiguousarray(biases[i]),
            }
        )
    return in_maps


def postprocess(res, B):
    return np.stack(
        [np.asarray(res.results[i]["out"]).astype(np.float32) for i in range(B)], axis=0
    )


def kernel(x, dw_kernels, pw_kernels, biases):
    B = np.asarray(x).shape[0]
    assert B == 8
    nc = get_nc()
    in_maps = make_in_maps(x, dw_kernels, pw_kernels, biases)
    res = run_bass_kernel_spmd(nc, in_maps, core_ids=list(range(B)))
    return postprocess(res, B)
